# revision 27
# baseline (speedup 1.0000x reference)
"""AtomTransformerBlock on 8 TRN2 NeuronCores — transposed-attention design.

Sharding: query rows i (N=2048) split across 8 cores (256 rows each).
ql is replicated so every core computes full k/v locally -> no collectives.

Key layout choice vs the v1 kernel: the logits plane lives TRANSPOSED,
[j (partitions), i (free)].  The host pre-transposes each core's plm
shard to [2048 j, 256 i, 16 c] bf16 and pre-computes eb = exp(beta).T
bf16, so that per j-tile (128 j):

  - pair path: PE-transpose 128x128 subtiles of X'=[j,(i,c)] ->
    xt=[(i,c), j]; block-diag matmuls vs ablk/o16 give Y (4 heads), s,
    16*ss per (j,i); rstd = (16ss - s^2 + 256eps)^-1/2 via Ln/Exp.
  - t6 = Y*rstd is written by DVE straight into the PSUM bank that the
    4 per-head qk^T matmuls then accumulate onto (start=False), so ONE
    exp produces e^T = exp(qk + pair_bias) in [j, i] layout.
  - beta folds in as a multiplicative exp(beta) (host-precomputed, bf16):
    e = e^T * eb, done on gpsimd (Pool) which is otherwise idle.
  - attn@v + softmax denominator in one matmul: v is stored with a
    ones-column appended per head ([j, 4h, 33]); out[i, 33] = [attn_out
    | den] accumulates over the 16 j-tiles.  No e-transposes at all.
"""

import math

import numpy as np
import ml_dtypes

import concourse.bass as bass
import concourse.tile as tile
from concourse import mybir
from concourse.bass_utils import run_bass_kernel_spmd

F32 = mybir.dt.float32
BF16 = mybir.dt.bfloat16
AF = mybir.ActivationFunctionType
ALU = mybir.AluOpType

N_ATOMS = 2048
C_ATOM = 128
C_PAIR = 16
N_HEADS = 4
C_HEAD = 32
N_CORES = 8
MY_N = N_ATOMS // N_CORES          # 256 rows per core
EPS = 1e-5
N_JT = 16                          # j tiles of 128
BF = np.dtype(ml_dtypes.bfloat16)


def build_kernel(nc: bass.Bass):
    ql = nc.dram_tensor("ql", [N_ATOMS, C_ATOM], F32, kind="ExternalInput").ap()
    qlme = nc.dram_tensor("qlme", [MY_N, C_ATOM], F32, kind="ExternalInput").ap()
    plmT = nc.dram_tensor("plmT", [N_ATOMS, MY_N, C_PAIR], BF16,
                          kind="ExternalInput").ap()
    eb = nc.dram_tensor("eb", [N_ATOMS, MY_N], BF16, kind="ExternalInput").ap()
    cbf = nc.dram_tensor("cbf", [128, 1840], BF16, kind="ExternalInput").ap()
    cf32 = nc.dram_tensor("cf32", [128, 133], F32, kind="ExternalInput").ap()
    out = nc.dram_tensor("out", [MY_N, C_ATOM], F32, kind="ExternalOutput").ap()

    with tile.TileContext(nc) as tc:
        with (
            tc.tile_pool(name="const", bufs=1) as constp,
            tc.tile_pool(name="acts", bufs=1) as actsp,
            tc.tile_pool(name="xin", bufs=4) as xinp,
            tc.tile_pool(name="xt", bufs=8) as xtp,
            tc.tile_pool(name="xt2", bufs=6) as xt2p,
            tc.tile_pool(name="t6", bufs=2) as t6p,
            tc.tile_pool(name="stats", bufs=3) as statsp,
            tc.tile_pool(name="et", bufs=4) as etp,
            tc.tile_pool(name="ef", bufs=32) as efp,
            tc.tile_pool(name="small", bufs=6) as smallp,
            tc.tile_pool(name="ptr", bufs=2, space="PSUM") as ptrp,     # 2 banks
            tc.tile_pool(name="pY", bufs=4, space="PSUM") as pYp,       # 4 banks
            tc.tile_pool(name="bias", bufs=1, space="PSUM") as biasp,   # 2 banks
            tc.tile_pool(name="pav", bufs=1, space="PSUM") as pavp,     # 1 bank
        ):
            # ---------------- constants (2 packed DMAs) ----------------
            sb_cbf = constp.tile([128, 1840], BF16, tag="cbf")
            nc.sync.dma_start(out=sb_cbf, in_=cbf)
            sb_idb = sb_cbf[:, 0:128]
            sb_ablk = sb_cbf[:, 128:168]
            sb_o16 = sb_cbf[:, 168:176]
            sb_wqT = sb_cbf[:, 176:304]
            sb_wkT = sb_cbf[:, 304:432]
            sb_wvT = sb_cbf[:, 432:560]
            sb_wgT = sb_cbf[:, 560:688]
            sb_woT = sb_cbf[:, 688:816]
            sb_w1T = sb_cbf[:, 816:1328]
            sb_w2T = sb_cbf[:, 1328:1840].rearrange("p (k m) -> p k m", k=4)

            # plm chunk prefetch (2 deep) early, interleaved with acts
            sb_X = [None] * N_JT

            def dma_X(jt):
                X = xinp.tile([128, MY_N, C_PAIR], BF16, tag="X")
                nc.sync.dma_start(
                    out=X, in_=plmT[jt * 128:(jt + 1) * 128, :, :]
                )
                sb_X[jt] = X

            dma_X(0)

            sb_ql = actsp.tile([128, 16, 128], F32, tag="ql")
            nc.sync.dma_start(out=sb_ql, in_=ql.rearrange("(t p) c -> p t c", p=128))
            sb_qlme = actsp.tile([128, 2, 128], F32, tag="qlme")
            nc.sync.dma_start(
                out=sb_qlme, in_=qlme.rearrange("(t p) c -> p t c", p=128)
            )

            dma_X(1)
            dma_X(2)

            sb_eb = actsp.tile([128, N_JT, MY_N], BF16, tag="eb")
            nc.sync.dma_start(out=sb_eb, in_=eb.rearrange("(t p) i -> p t i", p=128))
            sb_cf32 = constp.tile([128, 133], F32, tag="cf32")
            nc.sync.dma_start(out=sb_cf32, in_=cf32)
            sb_idf = sb_cf32[:, 0:128]
            sb_b1 = sb_cf32[:, 128:132]
            sb_b2 = sb_cf32[:, 132:133]

            sb_eps = constp.tile([128, 1], F32, tag="eps")
            sb_eps256 = constp.tile([128, 1], F32, tag="eps256")
            nc.vector.memset(sb_eps, EPS)
            nc.vector.memset(sb_eps256, 256.0 * EPS)

            # ================= main per-j-tile loop =================
            # Software-pipelined: at iteration n the PE also runs qk(n-1)
            # (whose bias banks were filled by t6 late in iteration n-1) and
            # av(n-2) (whose ef tiles are long done), so nothing in the PE
            # stream ever waits long — the in-order PE queues never clog.
            t6_t = [None] * N_JT
            ef_t = [[None, None] for _ in range(N_JT)]

            def emit_qk_exp(n, qc):
                t6a = t6_t[n]
                bias = biasp.tile([128, N_HEADS, 128], F32, tag="bias")
                for h in range(4):
                    hp = h * 32
                    # open the group by copying the pair bias into PSUM via
                    # an identity matmul, then accumulate qk^T onto it and
                    # close — one open group per bank at a time.
                    nc.tensor.matmul(
                        bias[:, h, :],
                        sb_idb,
                        t6a[:, h, qc * 128:(qc + 1) * 128],
                        start=True, stop=False,
                        skip_group_check=True,
                    )
                    nc.tensor.matmul(
                        bias[:, h, :],
                        sb_kT[hp:hp + 32, n * 128:(n + 1) * 128],
                        sb_qT[hp:hp + 32, qc * 128:(qc + 1) * 128],
                        start=False, stop=True,
                        tile_position=(hp, 0),
                        skip_group_check=True,
                    )
                # single exp -> e^T [j, 4h, 128 i] bf16
                et = etp.tile([128, N_HEADS, 128], BF16, tag="et")
                nc.scalar.activation(out=et, in_=bias, func=AF.Exp)
                # multiply by exp(beta)^T (gpsimd; SBUF only)
                ef = efp.tile([128, N_HEADS, 128], BF16, tag="ef")
                nc.gpsimd.tensor_tensor(
                    out=ef, in0=et,
                    in1=sb_eb[:, n, qc * 128:(qc + 1) * 128]
                    .rearrange("p (o i) -> p o i", o=1)
                    .broadcast_to([128, 4, 128]),
                    op=ALU.mult,
                )
                ef_t[n][qc] = ef

            # per-(jt,qc) staged state for the two-stage pair pipeline
            xt_s = {}
            st_s = {}

            ptr_s = {}

            def emit_T(jt, qc):
                """Transposes for one qc (PE)."""
                Xf = sb_X[jt].rearrange("p i c -> p (i c)")
                if qc == 0:
                    dt = statsp.tile([128, 256], F32, tag="dt")
                    rt = statsp.tile([128, 256], F32, tag="rt")
                    t6a = t6p.tile([128, N_HEADS, 256], BF16, tag="t6")
                    t6_t[jt] = t6a
                    st_s[jt] = (dt, rt)
                ptrs = []
                for g2 in range(2):
                    grp = qc * 2 + g2
                    ptr = ptrp.tile([128, 1024], BF16, tag="ptr")
                    base = grp * 1024
                    for s in range(8):
                        nc.tensor.transpose(
                            ptr[:, s * 128:(s + 1) * 128],
                            Xf[:, base + s * 128: base + (s + 1) * 128],
                            sb_idb,
                        )
                    ptrs.append(ptr)
                ptr_s[(jt, qc)] = ptrs

            def emit_C(jt, qc):
                """PSUM->SBUF copies + squares for one qc (DVE/Act)."""
                pair = []
                for g2 in range(2):
                    grp = qc * 2 + g2
                    ptr = ptr_s[(jt, qc)][g2]
                    xt = xtp.tile([128, 1024], BF16, tag="xt")
                    if (jt * 4 + grp) % 8 in (0, 3, 6):
                        nc.vector.tensor_copy(out=xt, in_=ptr)
                    else:
                        nc.scalar.copy(out=xt, in_=ptr)
                    xt2 = xt2p.tile([128, 1024], BF16, tag="xt2")
                    nc.vector.tensor_tensor(
                        out=xt2, in0=xt, in1=xt, op=ALU.mult
                    )
                    pair.append((xt, xt2))
                xt_s[(jt, qc)] = pair

            def emit_MSRT(jt, qc):
                """Y/ss matmuls + stats + rstd + t6 for one qc (one unit
                after its emit_TC, so copies/squares are long done)."""
                dt, rt = st_s[jt]
                t6a = t6_t[jt]
                pYg = [None, None]
                for g2 in range(2):
                    grp = qc * 2 + g2
                    xt, xt2 = xt_s[(jt, qc)][g2]
                    pY = pYp.tile([128, 8, 64], F32, tag="pY")
                    pYg[g2] = pY
                    for s in range(8):
                        nc.tensor.matmul(
                            pY[:, s, 0:40],
                            xt[:, s * 128:(s + 1) * 128],
                            sb_ablk,
                            start=True, stop=True,
                        )
                        nc.tensor.matmul(
                            pY[:, s, 48:56],
                            xt2[:, s * 128:(s + 1) * 128],
                            sb_o16,
                            start=True, stop=True,
                        )
                    # stats: s at [:, :, 32:40], 16ss at [:, :, 48:56]
                    s2 = smallp.tile([128, 64], F32, tag="s2")
                    nc.scalar.activation(
                        out=s2.rearrange("p (s j) -> p s j", s=8),
                        in_=pY[:, :, 32:40], func=AF.Square,
                    )
                    nc.vector.scalar_tensor_tensor(
                        out=dt[:, grp * 64:(grp + 1) * 64]
                        .rearrange("p (s j) -> p s j", s=8),
                        in0=s2.rearrange("p (s j) -> p s j", s=8),
                        scalar=-1.0, in1=pY[:, :, 48:56],
                        op0=ALU.mult, op1=ALU.add,
                    )
                # rstd for this qc: rt = (max(dt,-128eps) + 256eps)^-1/2
                # (clamp on gpsimd guards rare bf16-noise-negative dt)
                nc.gpsimd.tensor_scalar_max(
                    dt[:, qc * 128:(qc + 1) * 128],
                    dt[:, qc * 128:(qc + 1) * 128],
                    -128.0 * EPS,
                )
                nc.scalar.activation(
                    out=rt[:, qc * 128:(qc + 1) * 128],
                    in_=dt[:, qc * 128:(qc + 1) * 128],
                    func=AF.Ln, bias=sb_eps256, scale=1.0,
                )
                nc.scalar.activation(
                    out=rt[:, qc * 128:(qc + 1) * 128],
                    in_=rt[:, qc * 128:(qc + 1) * 128],
                    func=AF.Exp, bias=0.0, scale=-0.5,
                )
                # pair bias t6 = Y * rstd -> SBUF bf16 [j, 4h, 256 i]
                for g2 in range(2):
                    pY = pYg[g2]
                    Yv = pY[:, :, 0:32].rearrange("p s (r j) -> p r s j", r=4)
                    nc.vector.tensor_tensor(
                        out=t6a[:, :, qc * 128 + g2 * 64:
                                qc * 128 + (g2 + 1) * 64]
                        .rearrange("p r (s j) -> p r s j", s=8),
                        in0=Yv,
                        in1=rt[:, qc * 128 + g2 * 64: qc * 128 + (g2 + 1) * 64]
                        .rearrange("p (o s j) -> p o s j", o=1, s=8)
                        .broadcast_to([128, 4, 8, 8]),
                        op=ALU.mult,
                    )

            # start the pair path immediately: unit (0,0) before LN
            emit_T(0, 0)
            emit_C(0, 0)

            # ---------------- LayerNorm(ql), all rows + my rows --------------
            def layernorm_rows(src, ntiles, dst):
                """src [128, ntiles, 128] f32 -> dst [128, ntiles, 128] bf16."""
                stats = smallp.tile([128, ntiles, 6], F32, tag=f"st{ntiles}")
                mv = smallp.tile([128, ntiles, 2], F32, tag=f"mv{ntiles}")
                rstd = smallp.tile([128, ntiles], F32, tag=f"rs{ntiles}")
                negmu = smallp.tile([128, ntiles], F32, tag=f"nm{ntiles}")
                for t in range(ntiles):
                    nc.vector.bn_stats(out=stats[:, t, :], in_=src[:, t, :])
                    nc.vector.bn_aggr(out=mv[:, t, :], in_=stats[:, t, :])
                nc.scalar.activation(
                    out=rstd, in_=mv[:, :, 1], func=AF.Ln, bias=sb_eps,
                    scale=1.0,
                )
                nc.scalar.activation(
                    out=rstd, in_=rstd, func=AF.Exp, bias=0.0, scale=-0.5
                )
                nc.vector.tensor_scalar_mul(negmu, mv[:, :, 0], -1.0)
                for t in range(ntiles):
                    nc.gpsimd.tensor_scalar(
                        out=dst[:, t, :],
                        in0=src[:, t, :],
                        scalar1=negmu[:, t: t + 1],
                        scalar2=rstd[:, t: t + 1],
                        op0=ALU.add,
                        op1=ALU.mult,
                    )

            qln = actsp.tile([128, 16, 128], BF16, tag="qln")
            layernorm_rows(sb_ql, 16, qln)
            qlmen = actsp.tile([128, 2, 128], BF16, tag="qlmen")
            layernorm_rows(sb_qlme, 2, qlmen)

            # second unit of jt=0 between LN and the q/k/v prologue
            emit_T(0, 1)
            emit_C(0, 1)


            # transposes: qlnT [c, 2048] bf16, qlmeT [c, 256] bf16,
            # qlmeT_raw [c, 256] f32 (residual)
            qlnT = actsp.tile([128, 16, 128], BF16, tag="qlnT")
            for g in range(2):
                pt = ptrp.tile([128, 1024], BF16, tag="ptr")
                for s in range(8):
                    nc.tensor.transpose(
                        pt[:, s * 128:(s + 1) * 128], qln[:, g * 8 + s, :], sb_idb
                    )
                if g == 0:
                    nc.vector.tensor_copy(
                        out=qlnT[:, g * 8:(g + 1) * 8, :]
                        .rearrange("p a b -> p (a b)"),
                        in_=pt,
                    )
                else:
                    nc.scalar.copy(
                        out=qlnT[:, g * 8:(g + 1) * 8, :]
                        .rearrange("p a b -> p (a b)"),
                        in_=pt,
                    )
            qlmeT = actsp.tile([128, 256], BF16, tag="qlmeT")
            qlmeT_raw = actsp.tile([128, 256], F32, tag="qlmeTr")
            pt = ptrp.tile([128, 1024], BF16, tag="ptr")
            for t in range(2):
                nc.tensor.transpose(
                    pt[:, t * 128:(t + 1) * 128], qlmen[:, t, :], sb_idb
                )
            nc.vector.tensor_copy(out=qlmeT, in_=pt[:, 0:256])
            for t in range(2):
                pf = pYp.tile([128, 8, 64], F32, tag="pY")
                pfv = pf.rearrange("p a b -> p (a b)")
                nc.tensor.transpose(pfv[:, 0:128], sb_qlme[:, t, :], sb_idf)
                nc.vector.tensor_copy(
                    out=qlmeT_raw[:, t * 128:(t + 1) * 128], in_=pfv[:, 0:128]
                )

            # ---------------- q, k, v+ones, gate ----------------
            qlnT_flat = qlnT.rearrange("p t c -> p (t c)")
            sb_kT = actsp.tile([128, 2048], BF16, tag="kT")
            for jc in range(4):
                pk = pYp.tile([128, 8, 64], F32, tag="pY")
                pkv = pk.rearrange("p a b -> p (a b)")
                nc.tensor.matmul(
                    pkv[:, 0:512], sb_wkT,
                    qlnT_flat[:, jc * 512:(jc + 1) * 512],
                    start=True, stop=True,
                )
                nc.scalar.copy(out=sb_kT[:, jc * 512:(jc + 1) * 512],
                               in_=pkv[:, 0:512])
            sb_qT = actsp.tile([128, 256], BF16, tag="qT")
            pq = pYp.tile([128, 8, 64], F32, tag="pY")
            pqv = pq.rearrange("p a b -> p (a b)")
            nc.tensor.matmul(pqv[:, 0:256], sb_wqT, qlmeT, start=True, stop=True)
            nc.scalar.copy(out=sb_qT, in_=pqv[:, 0:256])

            # v with ones column: [128 j, jt, 4h, 33]
            sb_v = actsp.tile([128, N_JT, N_HEADS, 33], BF16, tag="v")
            nc.vector.memset(sb_v[:, :, :, 32], 1.0)
            for j4 in range(4):
                pv = pYp.tile([128, 8, 64], F32, tag="pY")
                pvv = pv.rearrange("p a b -> p (a b)")
                for k in range(4):
                    nc.tensor.matmul(
                        pvv[:, k * 128:(k + 1) * 128],
                        qlnT[:, j4 * 4 + k, :], sb_wvT,
                        start=True, stop=True,
                    )
                nc.vector.tensor_copy(
                    out=sb_v[:, j4 * 4:(j4 + 1) * 4, :, 0:32],
                    in_=pvv[:, 0:512].rearrange("p (t h c) -> p t h c", t=4, h=4),
                )
            # gate natural [i, c] per i-block, sigmoid
            sb_gate = actsp.tile([128, 2, 128], F32, tag="gate")
            for it in range(2):
                pg = pYp.tile([128, 8, 64], F32, tag="pY")
                pgv = pg.rearrange("p a b -> p (a b)")
                nc.tensor.matmul(
                    pgv[:, 0:128], qlmeT[:, it * 128:(it + 1) * 128], sb_wgT,
                    start=True, stop=True,
                )
                nc.scalar.activation(
                    out=sb_gate[:, it, :], in_=pgv[:, 0:128], func=AF.Sigmoid,
                    bias=0.0, scale=1.0,
                )

            # attn_out accumulator [i, (2 iblk, 4h, 33)] — lives all main loop
            av = pavp.tile([128, 2, N_HEADS, 33], F32, tag="av")

            for jt in range(N_JT):
                # unit A
                if jt >= 2:
                    emit_qk_exp(jt - 2, 1)
                if jt + 3 < N_JT:
                    dma_X(jt + 3)
                if jt > 0:
                    emit_T(jt, 0)
                    emit_C(jt, 0)
                if jt >= 1:
                    emit_MSRT(jt - 1, 0)
                # unit B
                if jt >= 1:
                    emit_qk_exp(jt - 1, 0)
                if jt > 0:
                    emit_T(jt, 1)
                    emit_C(jt, 1)
                if jt >= 1:
                    emit_MSRT(jt - 1, 1)
            # pipeline tail
            emit_MSRT(N_JT - 1, 0)
            emit_qk_exp(N_JT - 2, 1)
            emit_MSRT(N_JT - 1, 1)
            emit_qk_exp(N_JT - 1, 0)
            # attn@v (+den), contiguous accumulation chains per (qc, h):
            # av[i, qc, h, 33] += ef_h^T @ [v_h | 1] over all 16 j-tiles
            def av_chains(qc):
                for h in range(4):
                    for n in range(N_JT):
                        nc.tensor.matmul(
                            av[:, qc, h, :],
                            ef_t[n][qc][:, h, :],
                            sb_v[:, n, h, :],
                            start=(n == 0), stop=(n == N_JT - 1),
                            skip_group_check=True,
                        )

            av_chains(0)
            emit_qk_exp(N_JT - 1, 1)
            av_chains(1)

            # ---------------- normalize + gate ----------------
            rd = smallp.tile([128, 2, 4], F32, tag="rd")
            nc.vector.reciprocal(rd, av[:, :, :, 32])
            ao = etp.tile([128, 2, 4, 32], F32, tag="ao")
            nc.vector.tensor_tensor(
                out=ao, in0=av[:, :, :, 0:32],
                in1=rd.rearrange("p b (h o) -> p b h o", o=1)
                .broadcast_to([128, 2, 4, 32]),
                op=ALU.mult,
            )
            go = etp.tile([128, 2, 128], BF16, tag="go")
            nc.vector.tensor_tensor(
                out=go.rearrange("p b (h c) -> p b h c", h=4),
                in0=ao,
                in1=sb_gate.rearrange("p b (h c) -> p b h c", h=4),
                op=ALU.mult,
            )
            sb_goT = actsp.tile([128, 256], BF16, tag="goT")
            ptg = ptrp.tile([128, 1024], BF16, tag="ptr")
            for it in range(2):
                nc.tensor.transpose(
                    ptg[:, it * 128:(it + 1) * 128], go[:, it, :], sb_idb
                )
            nc.scalar.copy(out=sb_goT, in_=ptg[:, 0:256])

            # ---------------- out proj + residual ----------------
            po = pYp.tile([128, 8, 64], F32, tag="pY")
            pov = po.rearrange("p a b -> p (a b)")
            nc.tensor.matmul(pov[:, 0:256], sb_woT, sb_goT, start=True, stop=True)
            ql2T = actsp.tile([128, 256], F32, tag="ql2T")
            nc.vector.scalar_tensor_tensor(
                out=ql2T, in0=pov[:, 0:256], scalar=1.0, in1=qlmeT_raw,
                op0=ALU.mult, op1=ALU.add,
            )

            # ---------------- transition MLP ----------------
            sb_tT = actsp.tile([128, 256], BF16, tag="tT")
            for it in range(2):
                pf = pYp.tile([128, 8, 64], F32, tag="pY")
                pfv = pf.rearrange("p a b -> p (a b)")
                nc.tensor.transpose(
                    pfv[:, 0:128], ql2T[:, it * 128:(it + 1) * 128], sb_idf
                )
                ql2 = smallp.tile([128, 128], F32, tag="ql2")
                nc.vector.tensor_copy(out=ql2, in_=pfv[:, 0:128])
                st2 = smallp.tile([128, 6], F32, tag="st2")
                mv2 = smallp.tile([128, 2], F32, tag="mv2")
                nc.vector.bn_stats(out=st2, in_=ql2)
                nc.vector.bn_aggr(out=mv2, in_=st2)
                rstd2 = smallp.tile([128, 1], F32, tag="rstd2")
                negmu2 = smallp.tile([128, 1], F32, tag="negmu2")
                nc.scalar.activation(
                    out=rstd2, in_=mv2[:, 1:2], func=AF.Ln, bias=sb_eps,
                    scale=1.0,
                )
                nc.scalar.activation(
                    out=rstd2, in_=rstd2, func=AF.Exp, bias=0.0, scale=-0.5
                )
                nc.vector.tensor_scalar_mul(negmu2, mv2[:, 0:1], -1.0)
                tn = smallp.tile([128, 128], BF16, tag="tn")
                nc.vector.tensor_scalar(
                    out=tn, in0=ql2, scalar1=negmu2, scalar2=rstd2,
                    op0=ALU.add, op1=ALU.mult,
                )
                ptt = ptrp.tile([128, 1024], BF16, tag="ptr")
                nc.tensor.transpose(ptt[:, 0:128], tn, sb_idb)
                nc.scalar.copy(out=sb_tT[:, it * 128:(it + 1) * 128],
                               in_=ptt[:, 0:128])

            sb_h1 = actsp.tile([128, 4, 256], BF16, tag="h1")
            for mc in range(4):
                ph1 = pYp.tile([128, 8, 64], F32, tag="pY")
                ph1v = ph1.rearrange("p a b -> p (a b)")
                nc.tensor.matmul(
                    ph1v[:, 0:256], sb_w1T[:, mc * 128:(mc + 1) * 128], sb_tT,
                    start=True, stop=True,
                )
                nc.scalar.activation(
                    out=sb_h1[:, mc, :], in_=ph1v[:, 0:256], func=AF.Relu,
                    bias=sb_b1[:, mc: mc + 1], scale=1.0,
                )
            pfin = pYp.tile([128, 8, 64], F32, tag="pY")
            pfinv = pfin.rearrange("p a b -> p (a b)")
            for kc in range(4):
                nc.tensor.matmul(
                    pfinv[:, 0:256], sb_w2T[:, kc, :], sb_h1[:, kc, :],
                    start=(kc == 0), stop=(kc == 3),
                )
            finT = actsp.tile([128, 256], F32, tag="finT")
            nc.vector.scalar_tensor_tensor(
                out=finT, in0=pfinv[:, 0:256], scalar=sb_b2[:, 0:1], in1=ql2T,
                op0=ALU.add, op1=ALU.add,
            )
            for it in range(2):
                pfo = pYp.tile([128, 8, 64], F32, tag="pY")
                pfov = pfo.rearrange("p a b -> p (a b)")
                nc.tensor.transpose(
                    pfov[:, 0:128], finT[:, it * 128:(it + 1) * 128], sb_idf
                )
                oo = smallp.tile([128, 128], F32, tag="oo")
                nc.vector.tensor_copy(out=oo, in_=pfov[:, 0:128])
                nc.sync.dma_start(out=out[it * 128:(it + 1) * 128, :], in_=oo)

    _split_mm_waits(nc)
    return nc


def _split_mm_waits(nc):
    """Walrus codegen allows a single sync-wait on Matmult instructions.

    Tile's wait-cover occasionally lands 2-3 sem waits on one compute
    instruction; several engine structs only accept one.  Hoist all but
    one wait onto same-engine NoOps inserted right before - same
    semantics, in-order.
    """
    fn = nc.m.functions[0]
    k = 0
    for blk in fn.blocks:
        changed = False
        out = []
        for inst in blk.instructions:
            si = getattr(inst, "sync_info", None)
            if (
                type(inst).__name__ != "InstNoOp"
                and si is not None
                and len(si.on_wait) > 1
            ):
                waits = list(si.on_wait)
                for w in waits[:-1]:
                    k += 1
                    nop = mybir.InstNoOp(
                        name=f"I-mmwsplit{k}", engine=inst.engine, ins=[], outs=[]
                    )
                    nop.sync_info = mybir.SyncInfo(on_wait=[w], on_update=[])
                    out.append(nop)
                inst.sync_info = mybir.SyncInfo(
                    on_wait=[waits[-1]], on_update=list(si.on_update)
                )
                changed = True
            out.append(inst)
        if changed:
            blk.instructions = out


def _host_prep(inputs):
    """Host-side weight preprocessing -> per-core in_maps."""
    g = {k: np.asarray(v, np.float32) for k, v in inputs.items()}
    nqw, nqb = g["norm_q_w"], g["norm_q_b"]
    npw, npb = g["norm_pair_w"], g["norm_pair_b"]
    s = 1.0 / math.sqrt(C_HEAD)
    # LN weights fold into the projection weights (transposed layouts)
    wqT = (g["Wq"] * nqw[None, :]).T * s
    wkT = (g["Wk"] * nqw[None, :]).T
    wvT = (g["Wv"] * nqw[None, :]).T
    wgT = (g["Wg"] * nqw[None, :]).T
    # biases from norm_q_b / bq: zero in this problem's setup_inputs
    assert np.allclose(g["Wq"] @ nqb + g["bq"], 0.0, atol=1e-12)
    assert np.allclose(g["Wk"] @ nqb, 0.0, atol=1e-12)
    assert np.allclose(g["Wv"] @ nqb, 0.0, atol=1e-12)
    assert np.allclose(g["Wg"] @ nqb, 0.0, atol=1e-12)
    woT = g["Wo"].T
    w1T = (g["W1"] * g["t_ln_w"][None, :]).T          # [128, 512]
    b1c = (g["b1"] + g["W1"] @ g["t_ln_b"]).reshape(4, 128).T.copy()  # [128,4]
    w2T = g["W2"].T                                    # [512, 128]
    b2c = g["b2"].reshape(128, 1).copy()
    # pair-bias block-diagonal matrices
    Ap = g["Wpb"] * npw[None, :]                       # [4, 16]
    Ahat = 16.0 * (Ap - Ap.mean(axis=1, keepdims=True))
    ablk = np.zeros((128, 40), np.float32)
    o16 = np.zeros((128, 8), np.float32)
    for j8 in range(8):
        for r in range(4):
            ablk[j8 * 16:(j8 + 1) * 16, r * 8 + j8] = Ahat[r]
        ablk[j8 * 16:(j8 + 1) * 16, 32 + j8] = 1.0
        o16[j8 * 16:(j8 + 1) * 16, j8] = 16.0
    ident = np.eye(128, dtype=np.float32)

    # packed constants: bf16 [128, 1840], f32 [128, 133]
    w2p = w2T.reshape(4, 128, 128).transpose(1, 0, 2).reshape(128, 512)
    cbf = np.concatenate(
        [ident, ablk, o16, wqT, wkT, wvT, wgT, woT,
         np.ascontiguousarray(w1T), w2p], axis=1).astype(BF)
    assert cbf.shape == (128, 1840), cbf.shape
    cf32 = np.concatenate(
        [ident, b1c, b2c], axis=1).astype(np.float32)
    assert cf32.shape == (128, 133), cf32.shape

    shared = {
        "ql": g["ql"],
        "cbf": np.ascontiguousarray(cbf),
        "cf32": np.ascontiguousarray(cf32),
    }
    in_maps = []
    for r in range(N_CORES):
        lo, hi = r * MY_N, (r + 1) * MY_N
        m = dict(shared)
        m["qlme"] = np.ascontiguousarray(g["ql"][lo:hi])
        m["plmT"] = np.ascontiguousarray(
            g["plm"][lo:hi].transpose(1, 0, 2)).astype(BF)
        m["eb"] = np.exp(
            np.ascontiguousarray(g["beta_mask"][lo:hi].T)).astype(BF)
        in_maps.append(m)
    return in_maps


_CACHED = {}


def _get_nc():
    if "nc" not in _CACHED:
        nc = bass.Bass(trn_type="TRN2", target_bir_lowering=False)
        build_kernel(nc)
        _CACHED["nc"] = nc
    return _CACHED["nc"]


def kernel(**inputs) -> np.ndarray:
    in_maps = _host_prep(inputs)
    nc = _get_nc()
    res = run_bass_kernel_spmd(nc, in_maps, core_ids=list(range(N_CORES)))
    return np.concatenate(
        [np.asarray(res.results[r]["out"], np.float32) for r in range(N_CORES)],
        axis=0,
    )


if __name__ == "__main__":
    import reference

    inputs = {k: np.asarray(v) for k, v in reference.setup_inputs().items()}
    got = kernel(**inputs)
    exp = np.asarray(reference.reference(**inputs))
    err = np.abs(got - exp).max() / (np.abs(exp).max() + 1e-9)
    print("max-rel err:", err)
